# Initial kernel scaffold
#
"""Multi-head attention block (RMSNorm QK + RoPE + GQA + softmax + O-proj)
on 8 Trainium2 NeuronCores.

Sharding: data parallel over batch (B=2) x tensor parallel over kv-head
quarters (NKV=8 -> 2 kv heads / 4 q heads per core). Each core computes a
partial output [S, HID] = attn_out_local @ Wo_rows(local heads); the host
sums the 4 partials per batch.

Device pipeline per core (all matmuls in float32r: full PE rate, ~1e-4 rel):
  stage 1: QKV projection (s-layout), per-head RMSNorm+RoPE fused on
           DVE/ACT reading PSUM, PE-transpose of q/k -> qT/kT [HD, S];
           v kept in s-layout [S, HD] per kv head.
  stage 2: per (q-chunk 512, head): scores^T tiles [k,q] = kT_tile.T@qT,
           exp on ACT (scale=1/sqrt(HD), no max subtraction -- RMSNorm
           bounds |scores| <= sqrt(HD)), PV accumulation [d, q] over
           k-tiles, denominator via ones-matmul [1, q]; normalize with
           DVE mul by gpsimd-broadcast reciprocal; O-proj contracts the
           4 local heads into out [S, HID] partial.

RoPE tables cos/sin (with rot-half sign fold and q/k norm weights folded
in) are computed on host from position_ids -- they are replicated,
sharding_hint-style.
"""

import numpy as np

B, S, HID = 2, 2048, 2048
NH, NKV, HD = 16, 8, 128
EPS = 1e-6
THETA = 1000000.0
N_CORES = 8

P = 128
NT = S // P            # 16 s-tiles
KT = HID // P          # 16 hidden k-tiles
QC = 4                 # q chunks of 512
QW = S // QC           # 512
HEADS = NH // 4        # 4 q heads per core
KV = NKV // 4          # 2 kv heads per core
EC = 4                 # output e chunks of 512
EW = HID // EC         # 512
PV_DELAY = 3           # software pipeline depth for pv/den behind scores

_CACHE = {}


def _build():
    if "nc" in _CACHE:
        return _CACHE["nc"]
    import concourse.tile as tile
    import concourse.mybir as mybir
    from concourse import bacc

    f32 = mybir.dt.float32
    f32r = mybir.dt.float32r
    AF = mybir.ActivationFunctionType
    OP = mybir.AluOpType

    nc = bacc.Bacc("TRN2", target_bir_lowering=False, debug=False,
                   num_devices=N_CORES)

    hsT_d = nc.dram_tensor("hsT", [NT * P, KT * P], f32r, kind="ExternalInput").ap()
    wq_d = nc.dram_tensor("wq", [HID, HEADS * HD], f32r, kind="ExternalInput").ap()
    wkv_d = nc.dram_tensor("wkv", [HID, 2 * KV * HD], f32r, kind="ExternalInput").ap()
    wo_d = nc.dram_tensor("wo", [HEADS * HD, HID], f32r, kind="ExternalInput").ap()
    cq_d = nc.dram_tensor("cosq", [S, HD], f32, kind="ExternalInput").ap()
    sq_d = nc.dram_tensor("sinxq", [S, HD], f32, kind="ExternalInput").ap()
    ck_d = nc.dram_tensor("cosk", [S, HD], f32, kind="ExternalInput").ap()
    sk_d = nc.dram_tensor("sinxk", [S, HD], f32, kind="ExternalInput").ap()
    id_d = nc.dram_tensor("ident", [P, P], f32r, kind="ExternalInput").ap()
    on_d = nc.dram_tensor("ones", [P, 1], f32r, kind="ExternalInput").ap()
    out_d = nc.dram_tensor("out", [S, HID], f32, kind="ExternalOutput").ap()

    import math
    inv_sqrt_hd = 1.0 / math.sqrt(HD)

    with tile.TileContext(nc) as tc:
        with tc.tile_pool(name="const", bufs=1) as cpool, \
             tc.tile_pool(name="qkv", bufs=1) as qkv_pool:
            ident = cpool.tile([P, P], f32r)
            ones = cpool.tile([P, 1], f32r)
            epsb = cpool.tile([P, 1], f32)
            nc.sync.dma_start(ident[:], id_d[:])
            nc.sync.dma_start(ones[:], on_d[:])
            nc.vector.memset(epsb[:], EPS)

            qT = [qkv_pool.tile([P, S], f32r, tag=f"qT{h}", name=f"qT{h}") for h in range(HEADS)]
            kT = [qkv_pool.tile([P, S], f32r, tag=f"kT{j}", name=f"kT{j}") for j in range(KV)]
            vS = [qkv_pool.tile([P, S], f32r, tag=f"vS{j}", name=f"vS{j}") for j in range(KV)]

            # ---------------- stage 1: projections + norm + rope ----------
            with tc.tile_pool(name="w1", bufs=1) as w1, \
                 tc.tile_pool(name="tbl", bufs=1) as tbl, \
                 tc.tile_pool(name="hst", bufs=3) as hst, \
                 tc.tile_pool(name="rp", bufs=4) as rp, \
                 tc.tile_pool(name="sm", bufs=8) as sm, \
                 tc.tile_pool(name="ps_qkv", bufs=2, space="PSUM") as ps_qkv, \
                 tc.tile_pool(name="ps_tr", bufs=2, space="PSUM") as ps_tr:
                wq_sb = w1.tile([P, KT * HEADS * HD], f32r)
                wkv_sb = w1.tile([P, KT * 2 * KV * HD], f32r)
                cq_sb = tbl.tile([P, NT * HD], f32)
                sq_sb = tbl.tile([P, NT * HD], f32)
                ck_sb = tbl.tile([P, NT * HD], f32)
                sk_sb = tbl.tile([P, NT * HD], f32)
                NQ0 = HEADS * HD
                NKV0 = 2 * KV * HD
                wq_r = wq_d.rearrange("(k p) n -> p k n", p=P)
                wkv_r = wkv_d.rearrange("(k p) n -> p k n", p=P)
                for k in range(KT):
                    nc.sync.dma_start(wq_sb[:, k * NQ0:(k + 1) * NQ0],
                                      wq_r[:, k:k + 1, :])
                    nc.sync.dma_start(wkv_sb[:, k * NKV0:(k + 1) * NKV0],
                                      wkv_r[:, k:k + 1, :])
                for t in range(NT):
                    for sb_t, d_t in ((cq_sb, cq_d), (sq_sb, sq_d),
                                      (ck_sb, ck_d), (sk_sb, sk_d)):
                        nc.sync.dma_start(
                            sb_t[:, t * HD:(t + 1) * HD],
                            d_t.rearrange("(t p) d -> p t d", p=P)[:, t:t + 1, :])

                NQ = HEADS * HD   # 512
                NKVW = 2 * KV * HD  # 512 (k 0:256 | v 256:512)

                for t in range(NT):
                    hsTt = hst.tile([P, HID], f32r, tag="hsTt", name="hsTt")
                    nc.gpsimd.dma_start(hsTt[:], hsT_d[t * P:(t + 1) * P, :])
                    psq = ps_qkv.tile([P, NQ], f32, tag="psq")
                    pskv = ps_qkv.tile([P, NKVW], f32, tag="pskv")
                    for k in range(KT):
                        hk = hsTt[:, k * P:(k + 1) * P]
                        nc.tensor.matmul(psq[:], hk, wq_sb[:, k * NQ0:(k + 1) * NQ0],
                                         start=(k == 0), stop=(k == KT - 1))
                    for k in range(KT):
                        hk = hsTt[:, k * P:(k + 1) * P]
                        nc.tensor.matmul(pskv[:], hk, wkv_sb[:, k * NKV0:(k + 1) * NKV0],
                                         start=(k == 0), stop=(k == KT - 1))

                    NJ = HEADS + KV
                    def j_src(j):
                        if j < HEADS:
                            return psq[:, j * HD:(j + 1) * HD]
                        return pskv[:, (j - HEADS) * HD:(j - HEADS + 1) * HD]

                    sumsq = sm.tile([P, 8], f32, tag="sumsq")
                    for j in range(NJ):
                        sq_scr = rp.tile([P, HD], f32, tag="sq_scr")
                        nc.scalar.activation(sq_scr[:], j_src(j), AF.Square,
                                             accum_out=sumsq[:, j:j + 1])
                    std = sm.tile([P, 8], f32, tag="std")
                    nc.scalar.activation(std[:, 0:NJ], sumsq[:, 0:NJ], AF.Sqrt,
                                         scale=1.0 / HD, bias=epsb[:])
                    rstd = sm.tile([P, 8], f32, tag="rstd")
                    nc.vector.reciprocal(rstd[:, 0:NJ], std[:, 0:NJ])

                    ptr = ps_tr.tile([P, NJ * P], f32, tag="ptr")
                    for j in range(NJ):
                        src = j_src(j)
                        if j < HEADS:
                            cos_t = cq_sb[:, t * HD:(t + 1) * HD]
                            sin_t = sq_sb[:, t * HD:(t + 1) * HD]
                        else:
                            cos_t = ck_sb[:, t * HD:(t + 1) * HD]
                            sin_t = sk_sb[:, t * HD:(t + 1) * HD]
                        rs = rstd[:, j:j + 1]
                        t1 = rp.tile([P, HD], f32, tag="t1")
                        nc.vector.scalar_tensor_tensor(
                            t1[:], src, rs, cos_t, op0=OP.mult, op1=OP.mult)
                        t2 = rp.tile([P, HD], f32, tag="t2")
                        H2 = HD // 2
                        nc.vector.scalar_tensor_tensor(
                            t2[:, 0:H2], src[:, H2:HD], rs, sin_t[:, 0:H2],
                            op0=OP.mult, op1=OP.mult)
                        nc.vector.scalar_tensor_tensor(
                            t2[:, H2:HD], src[:, 0:H2], rs, sin_t[:, H2:HD],
                            op0=OP.mult, op1=OP.mult)
                        qrot = rp.tile([P, HD], f32r, tag="qrot")
                        nc.vector.tensor_add(qrot[:], t1[:], t2[:])
                        nc.tensor.transpose(
                            ptr[:, j * P:(j + 1) * P].bitcast(f32r), qrot[:], ident[:])

                    for h in range(HEADS):
                        nc.scalar.copy(qT[h][:, t * P:(t + 1) * P],
                                       ptr[:, h * P:(h + 1) * P])
                    for j in range(KV):
                        nc.scalar.copy(kT[j][:, t * P:(t + 1) * P],
                                       ptr[:, (HEADS + j) * P:(HEADS + j + 1) * P])
                        nc.scalar.copy(vS[j][:, t * P:(t + 1) * P],
                                       pskv[:, (KV + j) * HD:(KV + j + 1) * HD])

            # ---------------- stage 2: attention + O-projection -----------
            with tc.tile_pool(name="w2", bufs=1) as w2, \
                 tc.tile_pool(name="ep", bufs=10) as ep, \
                 tc.tile_pool(name="on", bufs=8) as on, \
                 tc.tile_pool(name="rc", bufs=4) as rc, \
                 tc.tile_pool(name="ob", bufs=4) as ob, \
                 tc.tile_pool(name="ps_a", bufs=4, space="PSUM") as ps_a, \
                 tc.tile_pool(name="ps_pv", bufs=2, space="PSUM") as ps_pv, \
                 tc.tile_pool(name="ps_dn", bufs=2, space="PSUM") as ps_dn:
                wo_sb = w2.tile([P, HEADS * HID], f32r)
                nc.sync.dma_start(wo_sb[:], wo_d.rearrange("(h p) e -> p h e", p=P))

                def emit_wo(c, onT_c):
                    for qt in range(QC):
                        out_row = ob.tile([P, HID], f32, tag="outb", name="out_row")
                        for e_ in range(EC):
                            pso = ps_a.tile([P, EW], f32, tag="att", name="pso")
                            for h in range(HEADS):
                                nc.tensor.matmul(
                                    pso[:],
                                    onT_c[h][:, qt * P:(qt + 1) * P],
                                    wo_sb[:, h * HID + e_ * EW: h * HID + (e_ + 1) * EW],
                                    start=(h == 0), stop=(h == HEADS - 1))
                            nc.vector.tensor_copy(out_row[:, e_ * EW:(e_ + 1) * EW],
                                                  pso[:])
                        nc.sync.dma_start(
                            out_d[c * QW + qt * P: c * QW + (qt + 1) * P, :],
                            out_row[:])

                prev_wo = None
                for c in range(QC):
                    onT_c = []
                    for h in range(HEADS):
                        kv = h // (HEADS // KV)
                        ppv = ps_pv.tile([P, QW], f32, tag="ppv", name="ppv")
                        pden = ps_dn.tile([1, QW], f32, tag="pden", name="pden")
                        pend = []
                        for k in range(NT):
                            psc = ps_a.tile([P, QW], f32, tag="att", name="psc")
                            nc.tensor.matmul(
                                psc[:], kT[kv][:, k * P:(k + 1) * P],
                                qT[h][:, c * QW:(c + 1) * QW],
                                start=True, stop=True)
                            e_t = ep.tile([P, QW], f32r, tag="e", name="e_t")
                            nc.scalar.activation(e_t[:], psc[:], AF.Exp,
                                                 scale=inv_sqrt_hd)
                            pend.append((k, e_t))
                            if len(pend) > PV_DELAY:
                                kk, ee = pend.pop(0)
                                nc.tensor.matmul(
                                    ppv[:], vS[kv][:, kk * P:(kk + 1) * P], ee[:],
                                    start=(kk == 0), stop=(kk == NT - 1))
                                nc.tensor.matmul(
                                    pden[:], ones[:], ee[:],
                                    start=(kk == 0), stop=(kk == NT - 1))
                            if h == 0 and k == 7 and prev_wo is not None:
                                # hide the previous chunk's O-projection inside
                                # this chunk's first score stream
                                emit_wo(*prev_wo)
                                prev_wo = None
                        for kk, ee in pend:
                            nc.tensor.matmul(
                                ppv[:], vS[kv][:, kk * P:(kk + 1) * P], ee[:],
                                start=(kk == 0), stop=(kk == NT - 1))
                            nc.tensor.matmul(
                                pden[:], ones[:], ee[:],
                                start=(kk == 0), stop=(kk == NT - 1))
                        rcp = rc.tile([1, QW], f32, tag="rcp", name="rcp")
                        nc.vector.reciprocal(rcp[:], pden[:])
                        bc = rc.tile([P, QW], f32, tag="bc", name="bc")
                        nc.gpsimd.partition_broadcast(bc[:], rcp[0:1, :])
                        onT = on.tile([P, QW], f32r, tag="onT", name="onT")
                        nc.vector.tensor_mul(onT[:], ppv[:], bc[:])
                        onT_c.append(onT)

                    prev_wo = (c, onT_c)
                emit_wo(*prev_wo)

    nc.compile()
    _CACHE["nc"] = nc
    return nc


def _host_prep(hidden_states, position_ids, Wq, Wk, Wv, Wo, q_norm_w, k_norm_w):
    """Build the 8 per-core input maps."""
    hidden_states = np.asarray(hidden_states, dtype=np.float32)
    Wq = np.asarray(Wq, dtype=np.float32)
    Wk = np.asarray(Wk, dtype=np.float32)
    Wv = np.asarray(Wv, dtype=np.float32)
    Wo = np.asarray(Wo, dtype=np.float32)
    q_norm_w = np.asarray(q_norm_w, dtype=np.float32)
    k_norm_w = np.asarray(k_norm_w, dtype=np.float32)
    pos = np.asarray(position_ids)

    ident = np.eye(P, dtype=np.float32)
    ones = np.ones((P, 1), dtype=np.float32)

    # per-batch rope tables with sign fold and norm-weight fold
    inv_freq = (1.0 / THETA ** (np.arange(0, HD, 2, dtype=np.float32) / HD)
                ).astype(np.float32)
    tabs = []
    H2 = HD // 2
    for b in range(B):
        freqs = pos[b].astype(np.float32)[:, None] * inv_freq[None, :]
        emb = np.concatenate([freqs, freqs], axis=-1)          # [S, HD]
        cos = np.cos(emb).astype(np.float32)
        sin = np.sin(emb).astype(np.float32)
        sinx = sin.copy()
        sinx[:, :H2] *= -1.0
        wq_sw = np.concatenate([q_norm_w[H2:], q_norm_w[:H2]])
        wk_sw = np.concatenate([k_norm_w[H2:], k_norm_w[:H2]])
        tabs.append({
            "cosq": np.ascontiguousarray(cos * q_norm_w[None, :]),
            "sinxq": np.ascontiguousarray(sinx * wq_sw[None, :]),
            "cosk": np.ascontiguousarray(cos * k_norm_w[None, :]),
            "sinxk": np.ascontiguousarray(sinx * wk_sw[None, :]),
        })

    # Pre-tiled transpose: hsT_t[t*P+p, k*P+c] = hs[b][t*P+c, k*P+p] so each
    # s-tile's SBUF load is a plain [P, HID] slice with 8KB-contiguous rows.
    hsT = []
    for b in range(B):
        x = hidden_states[b].reshape(NT, P, KT, P)      # [t, c, k, p]
        x = np.ascontiguousarray(x.transpose(0, 3, 2, 1))  # [t, p, k, c]
        hsT.append(x.reshape(NT * P, KT * P))

    in_maps = []
    for c in range(N_CORES):
        b = c // 4
        q = c % 4
        qs = slice(q * HEADS * HD, (q + 1) * HEADS * HD)
        ks = slice(q * KV * HD, (q + 1) * KV * HD)
        in_maps.append({
            "hsT": hsT[b],
            "wq": np.ascontiguousarray(Wq[:, qs]),
            "wkv": np.ascontiguousarray(
                np.concatenate([Wk[:, ks], Wv[:, ks]], axis=1)),
            "wo": np.ascontiguousarray(Wo[qs, :]),
            "cosq": tabs[b]["cosq"],
            "sinxq": tabs[b]["sinxq"],
            "cosk": tabs[b]["cosk"],
            "sinxk": tabs[b]["sinxk"],
            "ident": ident,
            "ones": ones,
        })
    return in_maps


def _gather(results):
    out = np.empty((B, S, HID), dtype=np.float32)
    for b in range(B):
        acc = results[4 * b]["out"].astype(np.float32)
        for i in range(1, 4):
            acc = acc + results[4 * b + i]["out"]
        out[b] = acc
    return out


def kernel(hidden_states, position_ids, Wq, Wk, Wv, Wo, q_norm_w, k_norm_w,
           _trace=False):
    from concourse.bass_utils import run_bass_kernel_spmd

    nc = _build()
    in_maps = _host_prep(hidden_states, position_ids, Wq, Wk, Wv, Wo,
                         q_norm_w, k_norm_w)
    res = run_bass_kernel_spmd(nc, in_maps, core_ids=list(range(N_CORES)),
                               trace=_trace)
    out = _gather(res.results)
    if _trace:
        kernel.last_result = res
    return out



# revision 27
# speedup vs baseline: 1.3640x; 1.3640x over previous
"""Multi-head attention block (RMSNorm QK + RoPE + GQA + softmax + O-proj)
on 8 Trainium2 NeuronCores.

Sharding: data parallel over batch (B=2) x tensor parallel over kv-head
quarters (NKV=8 -> 2 kv heads / 4 q heads per core). Each core computes a
partial output [S, HID] = attn_out_local @ Wo_rows(local heads); the host
sums the 4 partials per batch.

Device pipeline per core (all matmul operands bf16 -- streams ~216ns/512
cols vs ~278ns for f32r on TRN2 -- with fp32 PSUM accumulation):
  stage 1: QKV projection (s-layout), per-head RMSNorm+RoPE fused on
           DVE/ACT reading PSUM, PE-transpose of q/k -> qT/kT [HD, S]
           (f32r transposes, copies cast to bf16); v in s-layout per
           kv head.
  stage 2: per (q-chunk 512, head): scores^T tiles [k,q] = kT_tile.T@qT,
           exp on ACT (scale=1/sqrt(HD), no max subtraction -- RMSNorm
           bounds |scores| <= sqrt(HD)) -> bf16 e tiles; PV accumulation
           [d, q] over k-tiles; e accumulated into esum on DVE so the
           softmax denominator is a SINGLE ones-matmul per (head, chunk)
           (frees ~55us of PE vs per-k-tile den matmuls); normalize one
           head behind via reciprocal_approx_fast + gpsimd broadcast +
           DVE mul; O-proj contracts the 4 local heads into out [S, HID]
           partial, interleaved into the next chunk's score stream; out
           DMAs alternate sync/gpsimd queues to shorten the tail.

RoPE tables cos/sin (with rot-half sign fold and q/k norm weights folded
in) are computed on host from position_ids -- they are replicated,
sharding_hint-style.
"""

import numpy as np
import ml_dtypes

B, S, HID = 2, 2048, 2048
NH, NKV, HD = 16, 8, 128
EPS = 1e-6
THETA = 1000000.0
N_CORES = 8

P = 128
NT = S // P            # 16 s-tiles
KT = HID // P          # 16 hidden k-tiles
QC = 4                 # q chunks of 512
QW = S // QC           # 512
HEADS = NH // 4        # 4 q heads per core
KV = NKV // 4          # 2 kv heads per core
EC = 4                 # output e chunks of 512
EW = HID // EC         # 512
PV_DELAY = 4           # software pipeline depth for pv behind scores

_CACHE = {}


def _build():
    if "nc" in _CACHE:
        return _CACHE["nc"]
    import concourse.tile as tile
    import concourse.mybir as mybir
    from concourse import bacc

    f32 = mybir.dt.float32
    f32r = mybir.dt.float32r
    bf16 = mybir.dt.bfloat16
    AF = mybir.ActivationFunctionType
    OP = mybir.AluOpType

    nc = bacc.Bacc("TRN2", target_bir_lowering=False, debug=False,
                   num_devices=N_CORES)

    hsT_d = nc.dram_tensor("hsT", [NT * P, KT * P], bf16, kind="ExternalInput").ap()
    wq_d = nc.dram_tensor("wq", [HID, HEADS * HD], bf16, kind="ExternalInput").ap()
    wkv_d = nc.dram_tensor("wkv", [HID, 2 * KV * HD], bf16, kind="ExternalInput").ap()
    wo_d = nc.dram_tensor("wo", [HEADS * HD, HID], bf16, kind="ExternalInput").ap()
    cq_d = nc.dram_tensor("cosq", [S, HD], f32, kind="ExternalInput").ap()
    sq_d = nc.dram_tensor("sinxq", [S, HD], f32, kind="ExternalInput").ap()
    ck_d = nc.dram_tensor("cosk", [S, HD], f32, kind="ExternalInput").ap()
    sk_d = nc.dram_tensor("sinxk", [S, HD], f32, kind="ExternalInput").ap()
    id_d = nc.dram_tensor("ident", [P, P], f32r, kind="ExternalInput").ap()
    on_d = nc.dram_tensor("ones", [P, 1], bf16, kind="ExternalInput").ap()
    out_d = nc.dram_tensor("out", [S, HID], f32, kind="ExternalOutput").ap()

    import math
    inv_sqrt_hd = 1.0 / math.sqrt(HD)

    with tile.TileContext(nc) as tc:
        with tc.tile_pool(name="const", bufs=1) as cpool, \
             tc.tile_pool(name="qkv", bufs=1) as qkv_pool:
            ident = cpool.tile([P, P], f32r)
            ones = cpool.tile([P, 1], bf16)
            epsb = cpool.tile([P, 1], f32)
            nc.sync.dma_start(ident[:], id_d[:])
            nc.sync.dma_start(ones[:], on_d[:])
            nc.vector.memset(epsb[:], EPS)

            qT = [qkv_pool.tile([P, S], bf16, tag=f"qT{h}", name=f"qT{h}") for h in range(HEADS)]
            kT = [qkv_pool.tile([P, S], bf16, tag=f"kT{j}", name=f"kT{j}") for j in range(KV)]
            vS = [qkv_pool.tile([P, S], bf16, tag=f"vS{j}", name=f"vS{j}") for j in range(KV)]

            # ---------------- stage 1: projections + norm + rope ----------
            with tc.tile_pool(name="w1", bufs=1) as w1, \
                 tc.tile_pool(name="tbl", bufs=1) as tbl, \
                 tc.tile_pool(name="hst", bufs=4) as hst, \
                 tc.tile_pool(name="rp", bufs=4) as rp, \
                 tc.tile_pool(name="sm", bufs=8) as sm, \
                 tc.tile_pool(name="ps_qkv", bufs=2, space="PSUM") as ps_qkv, \
                 tc.tile_pool(name="ps_tr", bufs=2, space="PSUM") as ps_tr:
                wq_sb = w1.tile([P, KT * HEADS * HD], bf16)
                wkv_sb = w1.tile([P, KT * 2 * KV * HD], bf16)
                cq_sb = tbl.tile([P, NT * HD], f32)
                sq_sb = tbl.tile([P, NT * HD], f32)
                ck_sb = tbl.tile([P, NT * HD], f32)
                sk_sb = tbl.tile([P, NT * HD], f32)
                NQ0 = HEADS * HD
                NKV0 = 2 * KV * HD
                wq_r = wq_d.rearrange("(k p) n -> p k n", p=P)
                wkv_r = wkv_d.rearrange("(k p) n -> p k n", p=P)
                for k in range(KT):
                    nc.sync.dma_start(wq_sb[:, k * NQ0:(k + 1) * NQ0],
                                      wq_r[:, k:k + 1, :])
                    nc.sync.dma_start(wkv_sb[:, k * NKV0:(k + 1) * NKV0],
                                      wkv_r[:, k:k + 1, :])
                for t in range(NT):
                    for sb_t, d_t in ((cq_sb, cq_d), (sq_sb, sq_d),
                                      (ck_sb, ck_d), (sk_sb, sk_d)):
                        nc.sync.dma_start(
                            sb_t[:, t * HD:(t + 1) * HD],
                            d_t.rearrange("(t p) d -> p t d", p=P)[:, t:t + 1, :])

                NQ = HEADS * HD   # 512
                NKVW = 2 * KV * HD  # 512 (k 0:256 | v 256:512)

                for t in range(NT):
                    hsTt = hst.tile([P, HID], bf16, tag="hsTt", name="hsTt")
                    nc.gpsimd.dma_start(hsTt[:], hsT_d[t * P:(t + 1) * P, :])
                    psq = ps_qkv.tile([P, NQ], f32, tag="psq")
                    pskv = ps_qkv.tile([P, NKVW], f32, tag="pskv")
                    for k in range(KT):
                        hk = hsTt[:, k * P:(k + 1) * P]
                        nc.tensor.matmul(psq[:], hk, wq_sb[:, k * NQ0:(k + 1) * NQ0],
                                         start=(k == 0), stop=(k == KT - 1))
                    for k in range(KT):
                        hk = hsTt[:, k * P:(k + 1) * P]
                        nc.tensor.matmul(pskv[:], hk, wkv_sb[:, k * NKV0:(k + 1) * NKV0],
                                         start=(k == 0), stop=(k == KT - 1))

                    NJ = HEADS + KV
                    def j_src(j):
                        if j < HEADS:
                            return psq[:, j * HD:(j + 1) * HD]
                        return pskv[:, (j - HEADS) * HD:(j - HEADS + 1) * HD]

                    sumsq = sm.tile([P, 8], f32, tag="sumsq")
                    for j in range(NJ):
                        sq_scr = rp.tile([P, HD], f32, tag="sq_scr")
                        nc.scalar.activation(sq_scr[:], j_src(j), AF.Square,
                                             accum_out=sumsq[:, j:j + 1])
                    std = sm.tile([P, 8], f32, tag="std")
                    nc.scalar.activation(std[:, 0:NJ], sumsq[:, 0:NJ], AF.Sqrt,
                                         scale=1.0 / HD, bias=epsb[:])
                    rstd = sm.tile([P, 8], f32, tag="rstd")
                    nc.vector.reciprocal(rstd[:, 0:NJ], std[:, 0:NJ])

                    ptr = ps_tr.tile([P, NJ * P], f32, tag="ptr")
                    for j in range(NJ):
                        src = j_src(j)
                        if j < HEADS:
                            cos_t = cq_sb[:, t * HD:(t + 1) * HD]
                            sin_t = sq_sb[:, t * HD:(t + 1) * HD]
                        else:
                            cos_t = ck_sb[:, t * HD:(t + 1) * HD]
                            sin_t = sk_sb[:, t * HD:(t + 1) * HD]
                        rs = rstd[:, j:j + 1]
                        t1 = rp.tile([P, HD], f32, tag="t1")
                        nc.vector.scalar_tensor_tensor(
                            t1[:], src, rs, cos_t, op0=OP.mult, op1=OP.mult)
                        t2 = rp.tile([P, HD], f32, tag="t2")
                        H2 = HD // 2
                        nc.vector.scalar_tensor_tensor(
                            t2[:, 0:H2], src[:, H2:HD], rs, sin_t[:, 0:H2],
                            op0=OP.mult, op1=OP.mult)
                        nc.vector.scalar_tensor_tensor(
                            t2[:, H2:HD], src[:, 0:H2], rs, sin_t[:, H2:HD],
                            op0=OP.mult, op1=OP.mult)
                        qrot = rp.tile([P, HD], f32r, tag="qrot")
                        nc.vector.tensor_add(qrot[:], t1[:], t2[:])
                        nc.tensor.transpose(
                            ptr[:, j * P:(j + 1) * P].bitcast(f32r), qrot[:], ident[:])

                    for h in range(HEADS):
                        nc.scalar.copy(qT[h][:, t * P:(t + 1) * P],
                                       ptr[:, h * P:(h + 1) * P])
                    for j in range(KV):
                        nc.scalar.copy(kT[j][:, t * P:(t + 1) * P],
                                       ptr[:, (HEADS + j) * P:(HEADS + j + 1) * P])
                        nc.scalar.copy(vS[j][:, t * P:(t + 1) * P],
                                       pskv[:, (KV + j) * HD:(KV + j + 1) * HD])

            # ---------------- stage 2: attention + O-projection -----------
            with tc.tile_pool(name="w2", bufs=1) as w2, \
                 tc.tile_pool(name="ep", bufs=10) as ep, \
                 tc.tile_pool(name="es", bufs=2) as es, \
                 tc.tile_pool(name="on", bufs=8) as on, \
                 tc.tile_pool(name="rc", bufs=4) as rc, \
                 tc.tile_pool(name="ob", bufs=4) as ob, \
                 tc.tile_pool(name="ps_a", bufs=5, space="PSUM") as ps_a, \
                 tc.tile_pool(name="ps_pv", bufs=2, space="PSUM") as ps_pv, \
                 tc.tile_pool(name="ps_dn", bufs=1, space="PSUM") as ps_dn:
                wo_sb = w2.tile([P, HEADS * HID], bf16)
                nc.sync.dma_start(wo_sb[:], wo_d.rearrange("(h p) e -> p h e", p=P))

                def emit_wo_qt(c, onT_c, qt):
                    # one q-row-tile of the previous chunk's O-projection --
                    # interleaved per head so PE filler spreads evenly
                    out_row = ob.tile([P, HID], f32, tag="outb", name="out_row")
                    for e_ in range(EC):
                        pso = ps_a.tile([P, EW], f32, tag="att", name="pso")
                        for h in range(HEADS):
                            nc.tensor.matmul(
                                pso[:],
                                onT_c[h][:, qt * P:(qt + 1) * P],
                                wo_sb[:, h * HID + e_ * EW: h * HID + (e_ + 1) * EW],
                                start=(h == 0), stop=(h == HEADS - 1))
                        dst = out_row[:, e_ * EW:(e_ + 1) * EW]
                        nc.vector.tensor_copy(dst, pso[:])
                    eng = nc.sync if qt % 2 == 0 else nc.gpsimd
                    eng.dma_start(
                        out_d[c * QW + qt * P: c * QW + (qt + 1) * P, :],
                        out_row[:])

                def emit_norm(ppv, esum, onT):
                    # softmax denominator: one ones-matmul over the DVE-
                    # accumulated esum, then approx reciprocal + broadcast
                    pden = ps_dn.tile([1, QW], f32, tag="pden", name="pden")
                    nc.tensor.matmul(pden[:], ones[:], esum[:],
                                     start=True, stop=True)
                    sden = rc.tile([1, QW], f32, tag="sden", name="sden")
                    nc.vector.tensor_copy(sden[:], pden[:])
                    rcp = rc.tile([1, QW], f32, tag="rcp", name="rcp")
                    nc.vector.reciprocal_approx_fast(rcp[:], sden[:])
                    bc = rc.tile([P, QW], f32, tag="bc", name="bc")
                    nc.gpsimd.partition_broadcast(bc[:], rcp[0:1, :])
                    nc.vector.tensor_mul(onT[:], ppv[:], bc[:])

                prev_wo = None
                pending_norm = None
                onT_c = []
                for c in range(QC):
                    for h in range(HEADS):
                        kv = h // (HEADS // KV)
                        ppv = ps_pv.tile([P, QW], f32, tag="ppv", name="ppv")
                        esum = es.tile([P, QW], bf16, tag="esum", name="esum")
                        onT = on.tile([P, QW], bf16, tag="onT", name="onT")
                        pend = []
                        for k in range(NT):
                            psc = ps_a.tile([P, QW], f32, tag="att", name="psc")
                            nc.tensor.matmul(
                                psc[:], kT[kv][:, k * P:(k + 1) * P],
                                qT[h][:, c * QW:(c + 1) * QW],
                                start=True, stop=True)
                            e_t = ep.tile([P, QW], bf16, tag="e", name="e_t")
                            nc.scalar.activation(e_t[:], psc[:], AF.Exp,
                                                 scale=inv_sqrt_hd)
                            if k == 0:
                                nc.vector.tensor_copy(esum[:], e_t[:])
                            else:
                                nc.vector.tensor_add(esum[:], esum[:], e_t[:])
                            pend.append((k, e_t))
                            if len(pend) > PV_DELAY:
                                kk, ee = pend.pop(0)
                                nc.tensor.matmul(
                                    ppv[:], vS[kv][:, kk * P:(kk + 1) * P], ee[:],
                                    start=(kk == 0), stop=(kk == NT - 1))
                            if k == 2 and pending_norm is not None:
                                # normalize the previous head one head behind,
                                # inside this head's score stream
                                emit_norm(*pending_norm)
                                pending_norm = None
                            if prev_wo is not None and \
                                    k == (10 if h == 0 else 7):
                                # one quarter of the previous chunk's O-proj
                                # per head (h0 later: waits on its last norm)
                                emit_wo_qt(prev_wo[0], prev_wo[1], h)
                                if h == HEADS - 1:
                                    prev_wo = None
                        for kk, ee in pend:
                            nc.tensor.matmul(
                                ppv[:], vS[kv][:, kk * P:(kk + 1) * P], ee[:],
                                start=(kk == 0), stop=(kk == NT - 1))
                        pending_norm = (ppv, esum, onT)
                        onT_c.append(onT)
                    if c < QC - 1:
                        prev_wo = (c, onT_c)
                        onT_c = []
                emit_norm(*pending_norm)
                for qt in range(QC):
                    emit_wo_qt(QC - 1, onT_c, qt)

    nc.compile()
    _CACHE["nc"] = nc
    return nc


def _host_prep(hidden_states, position_ids, Wq, Wk, Wv, Wo, q_norm_w, k_norm_w):
    """Build the 8 per-core input maps."""
    hidden_states = np.asarray(hidden_states, dtype=np.float32)
    Wq = np.asarray(Wq, dtype=np.float32)
    Wk = np.asarray(Wk, dtype=np.float32)
    Wv = np.asarray(Wv, dtype=np.float32)
    Wo = np.asarray(Wo, dtype=np.float32)
    q_norm_w = np.asarray(q_norm_w, dtype=np.float32)
    k_norm_w = np.asarray(k_norm_w, dtype=np.float32)
    pos = np.asarray(position_ids)

    ident = np.eye(P, dtype=np.float32)
    ones = np.ones((P, 1), dtype=np.float32).astype(ml_dtypes.bfloat16)

    # per-batch rope tables with sign fold and norm-weight fold
    inv_freq = (1.0 / THETA ** (np.arange(0, HD, 2, dtype=np.float32) / HD)
                ).astype(np.float32)
    tabs = []
    H2 = HD // 2
    for b in range(B):
        freqs = pos[b].astype(np.float32)[:, None] * inv_freq[None, :]
        emb = np.concatenate([freqs, freqs], axis=-1)          # [S, HD]
        cos = np.cos(emb).astype(np.float32)
        sin = np.sin(emb).astype(np.float32)
        sinx = sin.copy()
        sinx[:, :H2] *= -1.0
        wq_sw = np.concatenate([q_norm_w[H2:], q_norm_w[:H2]])
        wk_sw = np.concatenate([k_norm_w[H2:], k_norm_w[:H2]])
        tabs.append({
            "cosq": np.ascontiguousarray(cos * q_norm_w[None, :]),
            "sinxq": np.ascontiguousarray(sinx * wq_sw[None, :]),
            "cosk": np.ascontiguousarray(cos * k_norm_w[None, :]),
            "sinxk": np.ascontiguousarray(sinx * wk_sw[None, :]),
        })

    # Pre-tiled transpose: hsT_t[t*P+p, k*P+c] = hs[b][t*P+c, k*P+p] so each
    # s-tile's SBUF load is a plain [P, HID] slice with 8KB-contiguous rows.
    hsT = []
    for b in range(B):
        x = hidden_states[b].reshape(NT, P, KT, P)      # [t, c, k, p]
        x = np.ascontiguousarray(x.transpose(0, 3, 2, 1))  # [t, p, k, c]
        hsT.append(x.reshape(NT * P, KT * P).astype(ml_dtypes.bfloat16))

    in_maps = []
    for c in range(N_CORES):
        b = c // 4
        q = c % 4
        qs = slice(q * HEADS * HD, (q + 1) * HEADS * HD)
        ks = slice(q * KV * HD, (q + 1) * KV * HD)
        in_maps.append({
            "hsT": hsT[b],
            "wq": np.ascontiguousarray(Wq[:, qs]).astype(ml_dtypes.bfloat16),
            "wkv": np.ascontiguousarray(
                np.concatenate([Wk[:, ks], Wv[:, ks]], axis=1)).astype(ml_dtypes.bfloat16),
            "wo": np.ascontiguousarray(Wo[qs, :]).astype(ml_dtypes.bfloat16),
            "cosq": tabs[b]["cosq"],
            "sinxq": tabs[b]["sinxq"],
            "cosk": tabs[b]["cosk"],
            "sinxk": tabs[b]["sinxk"],
            "ident": ident,
            "ones": ones,
        })
    return in_maps


def _gather(results):
    out = np.empty((B, S, HID), dtype=np.float32)
    for b in range(B):
        acc = results[4 * b]["out"].astype(np.float32)
        for i in range(1, 4):
            acc = acc + results[4 * b + i]["out"]
        out[b] = acc
    return out


def kernel(hidden_states, position_ids, Wq, Wk, Wv, Wo, q_norm_w, k_norm_w,
           _trace=False):
    from concourse.bass_utils import run_bass_kernel_spmd

    nc = _build()
    in_maps = _host_prep(hidden_states, position_ids, Wq, Wk, Wv, Wo,
                         q_norm_w, k_norm_w)
    res = run_bass_kernel_spmd(nc, in_maps, core_ids=list(range(N_CORES)),
                               trace=_trace)
    out = _gather(res.results)
    if _trace:
        kernel.last_result = res
    return out

